# revision 29
# baseline (speedup 1.0000x reference)
"""GCN layer (PyG GCNConv + PReLU) as a Trainium2 Bass kernel, SPMD over 8 NeuronCores.

Math (matching the reference):
    deg[c]  = in_degree(c) + 1          (over edge destinations)
    dis     = deg ** -0.5
    agg[c]  = sum_{e: col_e = c} dis[row_e] * x[row_e]     (self loops included)
    out[c]  = PReLU( (dis[c] * agg[c]) @ W + b )
The W transform is algebraically hoisted OUT of the edge aggregation
(segment_sum commutes with the dense matmul), so the device never
materializes per-edge transformed features.

Why this structure (from perfetto traces of a dma_gather-based variant):
the SWDGE dma_gather descriptor generation runs on a single Q7 CPU pair
at ~9 ns/index and instruction-serializes on the Pool engine (~2.4 ms
for 300k edge fetches) - far slower than just streaming the edge-
expanded rows at full DMA bandwidth.  So the gather moves to host layout
time: the host lays out, per core, the dis-scaled source rows of every
edge (Xg, bf16) in partition-major DRAM slabs the device can stream with
maximal-size descriptors.  The per-edge one-hot destination matrices S
come from two balanced sources: ~5/8 of tiles are built on the otherwise
idle vector engine (is_equal against an iota row, ~164 ns/tile, from a
64x-smaller crel metadata stream) and the rest stream as fp8 (0/1 exactly
representable) to keep both the DVE and the DMA engines at ~80% busy.
The aggregation runs entirely on the tensor engine:

    per dest block (128 dests):  aggT[f,d] = sum_t  Xg_t[slot,f]^T @ S_t[slot,d]
    final[d,o] = (aggT^T @ W)[d,o]   (bf16 inputs, fp32 PSUM)
    out = Prelu(final * dis[d])      (single fused scalar-engine op)

Edges are binned per (core, dest block); tiles-per-block is the max over
cores so all 8 cores share one program (~6% padding).  Padded slots have
S rows of zeros (crel = -1), so they contribute nothing.  The output is
written transposed ([128, NB*128], bf16) so each partition's slab is one
contiguous >=512B descriptor; the host un-transposes and upcasts.
"""

import math
import numpy as np

P = 128
D = 128
N_CORES = 8


# ----------------------------------------------------------------------------
# Host-side preparation: edge binning + partition-major slab layout
# ----------------------------------------------------------------------------

def _host_prep(x, edge_index, W, b, alpha, n_cores):
    import ml_dtypes

    x = np.ascontiguousarray(np.asarray(x, dtype=np.float32))
    ei = np.asarray(edge_index)
    W = np.asarray(W, dtype=np.float32)
    b = np.asarray(b, dtype=np.float32)
    alpha = np.asarray(alpha, dtype=np.float32)
    n_nodes = x.shape[0]
    src, col = ei[0].astype(np.int64), ei[1].astype(np.int64)

    shard = n_nodes // n_cores
    assert shard * n_cores == n_nodes
    NB = (shard + P - 1) // P

    deg = (np.bincount(col, minlength=n_nodes) + 1.0).astype(np.float32)
    dis = (1.0 / np.sqrt(deg)).astype(np.float32)

    # dis[src]-scaled features, quantized once to bf16
    xs = (x * dis[:, None]).astype(ml_dtypes.bfloat16)

    # self loops ride the main aggregation path
    loops = np.arange(n_nodes, dtype=np.int64)
    src = np.concatenate([src, loops])
    col = np.concatenate([col, loops])

    # per-(core, block) edge counts -> shared tiles-per-block schedule
    core_of = col // shard
    dloc = col - core_of * shard
    blk = dloc >> 7
    cnt = np.bincount(core_of * NB + blk, minlength=n_cores * NB)
    cnt = cnt.reshape(n_cores, NB)
    Tb = np.maximum((cnt.max(axis=0) + P - 1) // P, 1).astype(np.int64)
    tile_base = np.concatenate([[0], np.cumsum(Tb)])
    T_tot = int(tile_base[-1])

    uniform_alpha = bool(np.ptp(alpha) == 0.0)
    has_bias = bool(np.any(b != 0.0))

    # tile k's one-hot S comes from one of three sources by k % 32:
    #   r < DVE_N           -> built on the vector engine (is_equal, ~164ns)
    #   DVE_N <= r < SCAL_N -> built on the scalar engine (relu(1-|crel-iota|))
    #   else                -> streamed as pre-built fp8 over DMA
    DVE_N, SCAL_N, BUILD_DEN = 20, 22, 32
    kk = np.arange(T_tot)
    stream_tiles = np.nonzero((kk % BUILD_DEN) >= SCAL_N)[0]

    cfg = dict(
        shard=shard,
        nb=NB,
        Tb=[int(t) for t in Tb],
        T_tot=T_tot,
        n_stream=int(len(stream_tiles)),
        dve_n=DVE_N,
        scal_n=SCAL_N,
        build_den=BUILD_DEN,
        uniform_alpha=uniform_alpha,
        alpha0=float(alpha.flat[0]),
        has_bias=has_bias,
    )

    cores = []
    f8 = ml_dtypes.float8_e4m3
    for c in range(n_cores):
        lo = c * shard
        m = core_of == c
        s_c = src[m]
        d_c = dloc[m]
        b_c = blk[m]
        order = np.argsort(b_c, kind="stable")
        s_c, d_c, b_c = s_c[order], d_c[order], b_c[order]
        cnt_c = np.bincount(b_c, minlength=NB)
        off = np.concatenate([[0], np.cumsum(cnt_c)])[:-1]
        r = np.arange(len(s_c)) - off[b_c]
        tile_idx = tile_base[b_c] + (r >> 7)
        part = r & 127
        drel = d_c & 127

        Xg = np.zeros((P, T_tot, D), dtype=ml_dtypes.bfloat16)
        Xg[part, tile_idx, :] = xs[s_c]
        S = np.zeros((P, T_tot, P), dtype=f8)
        S[part, tile_idx, drel] = 1.0
        S = np.ascontiguousarray(S[:, stream_tiles, :])  # compact: streamed only
        crel = np.full((P, T_tot), -1.0, dtype=np.float32)
        crel[part, tile_idx] = drel.astype(np.float32)

        own = np.minimum(lo + np.arange(NB * P), n_nodes - 1)
        diso = dis[own].reshape(NB, P).T.copy()  # [P, NB]

        cores.append(dict(
            Xg=Xg.reshape(P, T_tot * D),
            S=S.reshape(P, len(stream_tiles) * P),
            crel=crel,
            diso=diso,
        ))

    shared = dict(
        W=W.astype(ml_dtypes.bfloat16),
        iota=np.broadcast_to(
            np.arange(P, dtype=np.float32), (P, P)
        ).astype(ml_dtypes.bfloat16),
    )
    if has_bias:
        shared["biasb"] = np.broadcast_to(b, (P, D)).copy()
    if not uniform_alpha:
        shared["alphab"] = np.broadcast_to(alpha, (P, D)).copy()
    return cfg, shared, cores


# ----------------------------------------------------------------------------
# Device program
# ----------------------------------------------------------------------------

def _build_program(cfg):
    import concourse.bass as bass
    import concourse.bacc as bacc
    import concourse.mybir as mybir
    import concourse.tile as tile
    from contextlib import ExitStack

    f32 = mybir.dt.float32
    bf16 = mybir.dt.bfloat16
    f8 = mybir.dt.float8e4
    AF = mybir.ActivationFunctionType
    OP = mybir.AluOpType

    NB = cfg["nb"]
    Tb = cfg["Tb"]
    T_tot = cfg["T_tot"]
    tile_base = [0]
    for t in Tb:
        tile_base.append(tile_base[-1] + t)

    # greedy-pack blocks into DMA slabs of at most CAP tiles
    CAP = 96
    groups = []  # list of (first_block, n_blocks, first_tile, n_tiles)
    bidx = 0
    while bidx < NB:
        b0 = bidx
        ntiles = 0
        while bidx < NB and ntiles + Tb[bidx] <= CAP:
            ntiles += Tb[bidx]
            bidx += 1
        groups.append((b0, bidx - b0, tile_base[b0], ntiles))

    # Per-tile interleave across three S sources (see _host_prep): keeps the
    # vector engine, scalar engine, and DMA engines all near-equally busy.
    DVE_N, SCAL_N, BUILD_DEN = cfg["dve_n"], cfg["scal_n"], cfg["build_den"]

    nc = bacc.Bacc()
    Xg = nc.declare_dram_parameter("Xg", [P, T_tot * D], bf16, isOutput=False)
    Sm = nc.declare_dram_parameter("S", [P, cfg["n_stream"] * P], f8, isOutput=False)
    crel = nc.declare_dram_parameter("crel", [P, T_tot], f32, isOutput=False)
    iota = nc.declare_dram_parameter("iota", [P, P], bf16, isOutput=False)
    Wp = nc.declare_dram_parameter("W", [P, D], bf16, isOutput=False)
    diso = nc.declare_dram_parameter("diso", [P, NB], f32, isOutput=False)
    if cfg["has_bias"]:
        biasb = nc.declare_dram_parameter("biasb", [P, D], f32, isOutput=False)
    if not cfg["uniform_alpha"]:
        alphab = nc.declare_dram_parameter("alphab", [P, D], f32, isOutput=False)
    # transposed output: out_pm[p, b*D + f] = out[b*P + p, f] (bf16, host upcasts)
    out = nc.declare_dram_parameter("out", [P, NB * D], bf16, isOutput=True)

    with tile.TileContext(nc) as tc, ExitStack() as ctx:
        const_p = ctx.enter_context(tc.tile_pool(name="const", bufs=1))
        W_sb = const_p.tile([P, D], bf16)
        nc.sync.dma_start(out=W_sb[:], in_=Wp[:])
        diso_sb = const_p.tile([P, NB], f32)
        nc.sync.dma_start(out=diso_sb[:], in_=diso[:])
        crel_sb = const_p.tile([P, T_tot], f32)
        nc.sync.dma_start(out=crel_sb[:], in_=crel[:])
        iota_sb = const_p.tile([P, P], bf16)
        nc.sync.dma_start(out=iota_sb[:], in_=iota[:])
        if cfg["has_bias"]:
            biasb_sb = const_p.tile([P, D], f32)
            nc.sync.dma_start(out=biasb_sb[:], in_=biasb[:])
        if not cfg["uniform_alpha"]:
            alphab_sb = const_p.tile([P, D], f32)
            nc.sync.dma_start(out=alphab_sb[:], in_=alphab[:])

        MAXBLK = max(
            nb_g for (_, nb_g, _, _) in groups
        )
        with (
            tc.tile_pool(name="xg", bufs=4) as xg_p,
            tc.tile_pool(name="ss", bufs=4) as s_p,
            tc.tile_pool(name="sb", bufs=24) as sb_p,
            tc.tile_pool(name="agg", bufs=4) as agg_p,
            tc.tile_pool(name="o", bufs=3) as o_p,
            tc.tile_pool(name="psA", bufs=4, space="PSUM") as psA_p,
            tc.tile_pool(name="psB", bufs=2, space="PSUM") as psB_p,
        ):
            soff = 0  # running index into the compacted stream-S tensor
            for gi, (b0, nb_g, t0, nt_g) in enumerate(groups):
                xg = xg_p.tile([P, CAP * D], bf16)
                nc.sync.dma_start(
                    out=xg[:, : nt_g * D], in_=Xg[:][:, t0 * D : (t0 + nt_g) * D]
                )
                sc = sum(
                    1 for k in range(t0, t0 + nt_g) if (k % BUILD_DEN) >= SCAL_N
                )
                if sc:
                    ss = s_p.tile([P, CAP * P], f8)
                    nc.sync.dma_start(
                        out=ss[:, : sc * P], in_=Sm[:][:, soff * P : (soff + sc) * P]
                    )
                og = o_p.tile([P, MAXBLK * P], bf16)
                sj = 0
                for bi in range(nb_g):
                    bb = b0 + bi
                    base = tile_base[bb] - t0
                    T = Tb[bb]
                    ps = psA_p.tile([P, P], f32)
                    for t in range(T):
                        k = base + t
                        r = (t0 + k) % BUILD_DEN
                        if r < DVE_N:
                            sbt = sb_p.tile([P, P], bf16)
                            nc.vector.tensor_scalar(
                                sbt[:], iota_sb[:],
                                crel_sb[:, t0 + k : t0 + k + 1],
                                None, OP.is_equal,
                            )
                            rhs_t = sbt[:]
                        elif r < SCAL_N:
                            # S = relu(1 - |crel - iota|): exact for integer
                            # crel/iota; pads (crel=-1) give all-zero rows
                            sct = sb_p.tile([P, P], bf16, tag="sc1")
                            nc.scalar.activation(
                                sct[:], iota_sb[:], AF.Abs,
                                bias=crel_sb[:, t0 + k : t0 + k + 1],
                                scale=-1.0,
                            )
                            sbt = sb_p.tile([P, P], bf16, tag="sc2")
                            nc.scalar.activation(
                                sbt[:], sct[:], AF.Relu, bias=1.0, scale=-1.0
                            )
                            rhs_t = sbt[:]
                        else:
                            rhs_t = ss[:, sj * P : (sj + 1) * P]
                            sj += 1
                        nc.tensor.matmul(
                            out=ps[:],
                            lhsT=xg[:, k * D : (k + 1) * D],
                            rhs=rhs_t,
                            start=(t == 0),
                            stop=(t == T - 1),
                        )
                    aggS = agg_p.tile([P, P], bf16)
                    nc.scalar.activation(aggS[:], ps[:], AF.Copy)
                    ps2 = psB_p.tile([P, P], f32)
                    nc.tensor.matmul(
                        out=ps2[:], lhsT=aggS[:], rhs=W_sb[:], start=True, stop=True
                    )
                    o = og[:, bi * P : (bi + 1) * P]
                    if cfg["uniform_alpha"] and not cfg["has_bias"]:
                        # out = Prelu(final * dis[dest]); dis > 0 commutes with PReLU
                        nc.scalar.activation(
                            o, ps2[:], AF.Prelu,
                            scale=diso_sb[:, bb : bb + 1],
                            alpha=cfg["alpha0"],
                        )
                    else:
                        pre = o_p.tile([P, P], f32, tag="pre")
                        nc.vector.tensor_scalar(
                            pre[:], ps2[:], diso_sb[:, bb : bb + 1], None, OP.mult
                        )
                        if cfg["has_bias"]:
                            nc.vector.tensor_tensor(
                                out=pre[:], in0=pre[:], in1=biasb_sb[:], op=OP.add
                            )
                        t1 = o_p.tile([P, P], f32, tag="t1")
                        nc.vector.tensor_scalar(t1[:], pre[:], 0.0, None, OP.max)
                        if cfg["uniform_alpha"]:
                            nc.vector.tensor_scalar(
                                o, pre[:], 0.0, cfg["alpha0"], OP.min, OP.mult
                            )
                        else:
                            nc.vector.tensor_scalar(o, pre[:], 0.0, None, OP.min)
                            nc.vector.tensor_tensor(
                                out=o, in0=o, in1=alphab_sb[:], op=OP.mult
                            )
                        nc.vector.tensor_tensor(out=o, in0=t1[:], in1=o, op=OP.add)
                assert sj == sc
                soff += sc
                nc.scalar.dma_start(
                    out=out[:][:, b0 * D : (b0 + nb_g) * D], in_=og[:, : nb_g * P]
                )
    nc.finalize()
    return nc


# ----------------------------------------------------------------------------
# Entry point
# ----------------------------------------------------------------------------

TRACE = False          # set True (e.g. from test.py) to capture an NTFF profile
LAST_RESULT = None     # BassKernelResults of the most recent kernel() call


def _install_ntff_hook():
    """Provide antenv.axon_hooks if the image lacks it (needed for trace=True)."""
    import sys, types
    try:
        from antenv import axon_hooks  # noqa: F401
        return
    except ImportError:
        pass
    try:
        import antenv
        from trn_agent_boot.trn_boot import _ntff_profile_via_ctypes
        hook = [_ntff_profile_via_ctypes("/opt/axon/libaxon_pjrt.so")]
    except Exception:
        return
    mod = types.ModuleType("antenv.axon_hooks")
    mod.set_axon_ntff_profile_hook = lambda h: hook.__setitem__(0, h)
    mod.get_axon_ntff_profile_hook = lambda: hook[0]
    sys.modules["antenv.axon_hooks"] = mod
    antenv.axon_hooks = mod


def kernel(x, edge_index, W, b, alpha):
    global LAST_RESULT
    if TRACE:
        _install_ntff_hook()
    from concourse.bass_utils import run_bass_kernel_spmd

    cfg, shared, cores = _host_prep(x, edge_index, W, b, alpha, N_CORES)
    nc = _build_program(cfg)
    in_maps = []
    for c in range(N_CORES):
        m = dict(shared)
        m.update(cores[c])
        in_maps.append(m)
    res = run_bass_kernel_spmd(nc, in_maps, list(range(N_CORES)), trace=TRACE)
    LAST_RESULT = res
    shard = cfg["shard"]
    NB = cfg["nb"]
    outs = []
    for c in range(N_CORES):
        o_pm = np.asarray(res.results[c]["out"]).astype(np.float32)  # [P, NB*D]
        o = o_pm.reshape(P, NB, D).transpose(1, 0, 2).reshape(NB * P, D)
        outs.append(o[:shard])
    return np.concatenate(outs, axis=0)
